# revision 1
# baseline (speedup 1.0000x reference)
"""Trainium2 Bass kernel for nn_ErrorAwareEdgeLoss.

reference:  cost[b,e] = sum_{p,q} P[b,i_e,p] * d_error[p,q] * P[b,j_e,q]
            result    = mean_{b,e} cost[b,e]

The edge pairs only enter through the count matrix
    C[l1,l2] = #edges e with (i_e,j_e) == (l1,l2),
and since d_error is symmetric the result collapses to
    result = <d_error, sum_b Q_b^T Cs Q_b> / (B*E),
with Q_b = P[b,:64,:] and Cs = (C + C^T)/2 (exact in bf16: half-integers).

Device work per core (256 batches, data-parallel over batch):
  - DMA P[b,:64,:] for 32 batches at a time into a [128, 2048] f32 tile
    (two batches stacked on the 128 partitions, 16 batch-pairs wide).
  - cast to bf16 (DVE)
  - Y = Cs @ Q via two concurrent 64x64-quadrant matmuls per 512-wide slab
  - R += QQ^T @ YY (K=128 contraction = 2 batches) accumulated in PSUM f32
  - write the per-core R (128x128 f32) to DRAM.
Host: R_total = sum_c R_c ;  result = <d_error, R_total> / (B*E) in f64.
"""

import sys

_TRN_REPO = "/opt/trn_rl_repo"
if _TRN_REPO not in sys.path:
    sys.path.insert(0, _TRN_REPO)

import numpy as np
import ml_dtypes

B, L, H = 2048, 64, 128     # batch, logical qubits, physical dim
E = 512                     # number of circuit edges
N_CORES = 8
BPC = B // N_CORES          # 256 batches per core
GROUP = 32                  # batches per DMA group
NGROUPS = BPC // GROUP      # 8
ROWS_PER_GROUP = GROUP * L  # 2048 DRAM rows per group
FREE = GROUP // 2 * H       # 2048 f32 per partition per group
SLAB = 512                  # matmul moving-operand width
NSLABS = FREE // SLAB       # 4
NBLK = GROUP // 2           # 16 K=128 blocks (2 batches each)

_CACHE = {}

# load variants:
#   "swdge_cast": one gpsimd DMA per group, f32->bf16 cast in the DMA
#   "hwdge_split": sync-engine f32 DMAs split into DMA_SPLIT pieces + DVE cast
LOAD_MODE = "swdge_cast"
DMA_SPLIT = 4
# group sizes in 8-batch slabs (sum must be 32 = 256 batches / 8).
# Small leading groups start the PE early (HWDGE path is up before the
# SWDGE rings are initialized); big trailing groups amortize DMA overhead.
GROUP_SLABS = [4, 4, 4, 4, 4, 4, 4, 4]
HWDGE_HEAD = 0  # this many leading groups load via sync f32 + DVE cast


def _build(load_mode=None):
    import concourse.tile as tile
    from concourse import bacc, mybir

    if load_mode is None:
        load_mode = LOAD_MODE
    f32 = mybir.dt.float32
    bf16 = mybir.dt.bfloat16

    nc = bacc.Bacc(None)
    # host-packed shard: row p holds, concatenated over (group, batch-pair),
    # the 128 floats of Q[g*32 + 2j + p//64, p%64, :] — so every per-group
    # load is a plain 2D DMA with an 8KB contiguous run per partition.
    pq = nc.dram_tensor("pq", [128, NGROUPS * FREE], f32, kind="ExternalInput")
    cs = nc.dram_tensor("cs", [L, L], bf16, kind="ExternalInput")
    r_out = nc.dram_tensor("r_out", [H, H], f32, kind="ExternalOutput")

    assert sum(GROUP_SLABS) * SLAB == NGROUPS * FREE

    with tile.TileContext(nc) as tc:
        with (
            tc.tile_pool(name="singles", bufs=1) as singles,
            tc.tile_pool(name="qraw", bufs=2) as qraw_pool,
            tc.tile_pool(name="qbfp", bufs=4) as qbf_pool,
            tc.tile_pool(name="ybfp", bufs=4) as ybf_pool,
            tc.tile_pool(name="yps", bufs=4, space="PSUM") as yps,
            tc.tile_pool(name="rps", bufs=1, space="PSUM") as rps,
        ):
            # Cs replicated into both partition halves so the two PE
            # quadrants (0,0) and (64,64) each see it as lhsT.
            cs2 = singles.tile([128, L], bf16)
            nc.sync.dma_start(out=cs2[0:64, :], in_=cs[:, :])
            nc.sync.dma_start(out=cs2[64:128, :], in_=cs[:, :])

            r_psum = rps.tile([128, H], f32)

            def load_group(gi, c0, width):
                qbf = qbf_pool.tile([128, width], bf16)
                if load_mode == "swdge_cast" and gi >= HWDGE_HEAD:
                    nc.gpsimd.dma_start(out=qbf[:, :], in_=pq[:, c0 : c0 + width])
                else:
                    qf32 = qraw_pool.tile([128, width], f32)
                    per = width // DMA_SPLIT if load_mode != "swdge_cast" else width
                    for dd in range(width // per):
                        nc.sync.dma_start(
                            out=qf32[:, dd * per : (dd + 1) * per],
                            in_=pq[:, c0 + dd * per : c0 + (dd + 1) * per],
                        )
                    nc.vector.tensor_copy(qbf[:, :], qf32[:, :])
                return qbf

            def emit_y(qbf, nslabs):
                ybf = ybf_pool.tile([128, nslabs * SLAB], bf16)
                for s in range(nslabs):
                    yy = yps.tile([128, SLAB], f32)
                    sl = slice(s * SLAB, (s + 1) * SLAB)
                    nc.tensor.matmul(
                        yy[0:64, :], lhsT=cs2[0:64, :], rhs=qbf[0:64, sl],
                        start=True, stop=True, skip_group_check=True,
                    )
                    nc.tensor.matmul(
                        yy[64:128, :], lhsT=cs2[64:128, :], rhs=qbf[64:128, sl],
                        start=True, stop=True, skip_group_check=True,
                    )
                    # PSUM -> SBUF cast copy, halves on DVE and ACT
                    half = SLAB // 2
                    nc.vector.tensor_copy(
                        ybf[:, s * SLAB : s * SLAB + half], yy[:, 0:half]
                    )
                    nc.scalar.copy(
                        ybf[:, s * SLAB + half : (s + 1) * SLAB],
                        yy[:, half:SLAB],
                    )
                return ybf

            _flags = {"first": True}

            def emit_r(qbf, ybf, nslabs, is_last_group):
                nblocks = nslabs * 4
                for k in range(nblocks):
                    first = _flags["first"]
                    _flags["first"] = False
                    last = is_last_group and k == nblocks - 1
                    nc.tensor.matmul(
                        r_psum[:, :],
                        lhsT=qbf[:, k * H : (k + 1) * H],
                        rhs=ybf[:, k * H : (k + 1) * H],
                        start=first, stop=last, skip_group_check=True,
                    )

            # Software pipeline: R-matmuls run one group behind the
            # Y-matmuls so the PE never waits on the PSUM->SBUF casts.
            prev = None
            c0 = 0
            for gi, k in enumerate(GROUP_SLABS):
                width = k * SLAB
                qbf = load_group(gi, c0, width)
                c0 += width
                ybf = emit_y(qbf, k)
                if prev is not None:
                    emit_r(*prev, is_last_group=False)
                prev = (qbf, ybf, k)
            emit_r(*prev, is_last_group=True)

            rsb = singles.tile([128, H], f32)
            nc.vector.tensor_copy(rsb[:, :], r_psum[:, :])
            nc.sync.dma_start(out=r_out[:, :], in_=rsb[:, :])

    nc.compile()
    return nc


def get_nc():
    key = ("nc", LOAD_MODE, DMA_SPLIT, tuple(GROUP_SLABS), HWDGE_HEAD)
    if key not in _CACHE:
        _CACHE[key] = _build()
    return _CACHE[key]


def make_count_matrix(circuit_edge_pairs):
    pairs = np.asarray(circuit_edge_pairs).astype(np.int64)
    C = np.zeros((L, L), np.float64)
    np.add.at(C, (pairs[:, 0], pairs[:, 1]), 1.0)
    Cs = (C + C.T) * 0.5
    return Cs.astype(ml_dtypes.bfloat16)


def pack_shard(Q):
    """(256, 64, 128) f32 -> (128, 16384) so partition p's per-group data
    T[p, g*2048 + j*128 + h] = Q[g*32 + 2j + p//64, p%64, h] is contiguous."""
    arr = Q.reshape(NGROUPS, GROUP // 2, 2, L, H)
    return np.ascontiguousarray(
        arr.transpose(2, 3, 0, 1, 4).reshape(128, NGROUPS * FREE)
    )


def make_in_maps(P, circuit_edge_pairs):
    P = np.asarray(P)
    csb = make_count_matrix(circuit_edge_pairs)
    in_maps = []
    for c in range(N_CORES):
        shard = np.ascontiguousarray(
            P[c * BPC : (c + 1) * BPC, :L, :], dtype=np.float32
        )
        in_maps.append({"pq": pack_shard(shard), "cs": csb})
    return in_maps


def reduce_results(per_core_r, d_error):
    R = np.zeros((H, H), np.float64)
    for r in per_core_r:
        R += np.asarray(r).astype(np.float64)
    out = (np.asarray(d_error).astype(np.float64) * R).sum() / (B * E)
    return np.array(out, dtype=np.float32)


def run_spmd(P, circuit_edge_pairs, **kwargs):
    """Run on the 8 NeuronCores; returns (list of per-core R, BassKernelResults)."""
    from concourse.bass_utils import run_bass_kernel_spmd

    nc = get_nc()
    in_maps = make_in_maps(P, circuit_edge_pairs)
    res = run_bass_kernel_spmd(nc, in_maps, core_ids=list(range(N_CORES)), **kwargs)
    per_core_r = [res.results[c]["r_out"] for c in range(N_CORES)]
    return per_core_r, res


def kernel(P, d_error, circuit_edge_pairs, num_logical):
    assert int(num_logical) == L
    per_core_r, _ = run_spmd(P, circuit_edge_pairs)
    return reduce_results(per_core_r, d_error)



# revision 2
# speedup vs baseline: 1.1171x; 1.1171x over previous
"""Trainium2 Bass kernel for nn_ErrorAwareEdgeLoss.

reference:  cost[b,e] = sum_{p,q} P[b,i_e,p] * d_error[p,q] * P[b,j_e,q]
            result    = mean_{b,e} cost[b,e]

The edge pairs only enter through the count matrix
    C2[l1,l2] = C + C^T,  C[l1,l2] = #edges with (i_e,j_e) == (l1,l2),
and since d_error is symmetric the result collapses to
    result = <d_error, sum_b Q_b^T C2 Q_b> / (2*B*E),
with Q_b = P[b,:64,:].  C2 has small integer entries (max ~6): exact in
bf16/fp8.

Device work per core (256 batches, data-parallel over batch):
  - HWDGE DMA of host-packed Q groups into [128, ncols] tiles
    (two batches stacked on the 128 partitions).
  - Y = blockdiag(C2,C2) @ Q: one K=128 matmul per 512-wide slab
    (the zero off-diagonal blocks cost nothing: matmul time = N cols).
  - PSUM->SBUF casts of Y, slab-granular, alternating DVE / ACT engines.
  - R += Q_pair^T @ Y_pair (K=128 = 2 batches, N=128) accumulated in
    PSUM f32; R-matmuls run one group behind Y so the PE never waits
    on the casts.
  - write per-core R (128x128 f32) to DRAM.
Host: R_total = sum_c R_c ; result = <d_error, R_total> / (2*B*E*S^2).
"""

import sys

_TRN_REPO = "/opt/trn_rl_repo"
if _TRN_REPO not in sys.path:
    sys.path.insert(0, _TRN_REPO)

import numpy as np
import ml_dtypes

B, L, H = 2048, 64, 128     # batch, logical qubits, physical dim
E = 512                     # number of circuit edges
N_CORES = 8
BPC = B // N_CORES          # 256 batches per core
SLAB = 512                  # matmul moving-operand width (= 4 pair-blocks)
NSLABS = BPC * L * H // (128 * SLAB)   # 32 slabs of 512 cols per core

# Q dtype: "bf16" (exact) or "fp8" (halves DMA traffic; rel err ~1.5e-3)
Q_DTYPE = "bf16"
Q_SCALE = 64.0 if Q_DTYPE == "fp8" else 1.0
# group sizes in 512-col slabs (sum must be NSLABS). Small leading groups
# start the PE early; bigger trailing groups amortize per-DMA overhead.
GROUP_SLABS = [2, 3, 4, 4, 4, 5, 5, 5]
# number of 512-col PE warmup matmuls issued before the first real slab
# (ramps the PE clock out of its low p-state while the first DMA lands)
WARMUP_MMS = 0

_CACHE = {}


def _np_qdt():
    return ml_dtypes.bfloat16 if Q_DTYPE == "bf16" else ml_dtypes.float8_e4m3fn


def _build():
    import concourse.tile as tile
    from concourse import bacc, mybir

    f32 = mybir.dt.float32
    bf16 = mybir.dt.bfloat16
    qdt = bf16 if Q_DTYPE == "bf16" else mybir.dt.float8e4

    assert sum(GROUP_SLABS) == NSLABS

    nc = bacc.Bacc(None)
    # host-packed shard: row p holds, concatenated over (slab, pair-block),
    # Q[batch, p%64, :] for batch = 2*(col block) + p//64 — every group
    # load is a plain 2D DMA with a contiguous run per partition.
    pq = nc.dram_tensor("pq", [128, NSLABS * SLAB], qdt, kind="ExternalInput")
    cs = nc.dram_tensor("cs", [L, L], qdt, kind="ExternalInput")
    r_out = nc.dram_tensor("r_out", [H, H], f32, kind="ExternalOutput")

    with tile.TileContext(nc) as tc:
        with (
            tc.tile_pool(name="singles", bufs=1) as singles,
            tc.tile_pool(name="qpool", bufs=4) as qpool,
            tc.tile_pool(name="ypool", bufs=4) as ypool,
            tc.tile_pool(name="yps", bufs=6, space="PSUM") as yps,
            tc.tile_pool(name="rps", bufs=1, space="PSUM") as rps,
        ):
            # block-diag(C2, C2) so one K=128 matmul transforms both batches
            # stacked on the partition dim.
            csbd = singles.tile([128, 128], qdt)
            nc.vector.memset(csbd[:, :], 0)
            nc.sync.dma_start(out=csbd[0:64, 0:64], in_=cs[:, :])
            nc.sync.dma_start(out=csbd[64:128, 64:128], in_=cs[:, :])

            r_psum = rps.tile([128, H], f32)

            if WARMUP_MMS:
                warm = singles.tile([128, SLAB], qdt)
                nc.vector.memset(warm[:, :], 0)
                wps = rps.tile([128, SLAB], f32)
                for _ in range(WARMUP_MMS):
                    nc.tensor.matmul(
                        wps[:, :], lhsT=csbd[:, :], rhs=warm[:, :],
                        start=True, stop=True, skip_group_check=True,
                    )

            _flags = {"first": True}

            def emit_group(gi, c0, k):
                npairs = 4 * k
                qbf = qpool.tile([128, npairs, H], qdt)
                nc.sync.dma_start(out=qbf[:, :, :], in_=pq[:, c0 : c0 + k * SLAB])
                ybf = ypool.tile([128, npairs, H], bf16)
                for s in range(k):
                    yy = yps.tile([128, SLAB], f32)
                    nc.tensor.matmul(
                        yy[:, :], lhsT=csbd[:, :], rhs=qbf[:, 4 * s : 4 * s + 4, :],
                        start=True, stop=True, skip_group_check=True,
                    )
                    # PSUM -> SBUF cast, alternating engines per slab
                    dst = ybf[:, 4 * s : 4 * s + 4, :]
                    if s % 2 == 0:
                        nc.vector.tensor_copy(dst, yy[:, :])
                    else:
                        nc.scalar.copy(dst, yy[:, :])
                return qbf, ybf, npairs

            def emit_r(qbf, ybf, npairs, is_last_group):
                for pp in range(npairs):
                    first = _flags["first"]
                    _flags["first"] = False
                    last = is_last_group and pp == npairs - 1
                    nc.tensor.matmul(
                        r_psum[:, :],
                        lhsT=qbf[:, pp, :],
                        rhs=ybf[:, pp, :],
                        start=first, stop=last, skip_group_check=True,
                    )

            prev = None
            c0 = 0
            for gi, k in enumerate(GROUP_SLABS):
                cur = emit_group(gi, c0, k)
                c0 += k * SLAB
                if prev is not None:
                    emit_r(*prev, is_last_group=False)
                prev = cur
            emit_r(*prev, is_last_group=True)

            rsb = singles.tile([128, H], f32)
            nc.vector.tensor_copy(rsb[:, :], r_psum[:, :])
            nc.sync.dma_start(out=r_out[:, :], in_=rsb[:, :])

    nc.compile()
    return nc


def get_nc():
    key = ("nc", Q_DTYPE, tuple(GROUP_SLABS), WARMUP_MMS)
    if key not in _CACHE:
        _CACHE[key] = _build()
    return _CACHE[key]


def make_count_matrix(circuit_edge_pairs):
    pairs = np.asarray(circuit_edge_pairs).astype(np.int64)
    C = np.zeros((L, L), np.float64)
    np.add.at(C, (pairs[:, 0], pairs[:, 1]), 1.0)
    C2 = C + C.T
    out = C2.astype(_np_qdt())
    assert np.array_equal(out.astype(np.float64), C2), "C2 not exact in Q dtype"
    return out


def pack_shard(Q):
    """(256, 64, 128) f32 -> (128, 16384): partition p, col j*128+h holds
    Q[2*j + p//64, p%64, h] * Q_SCALE (pair-blocked along columns)."""
    arr = (Q * Q_SCALE).reshape(BPC // 2, 2, L, H)
    return np.ascontiguousarray(
        arr.transpose(1, 2, 0, 3).reshape(128, NSLABS * SLAB)
    ).astype(_np_qdt())


def make_in_maps(P, circuit_edge_pairs):
    P = np.asarray(P)
    csb = make_count_matrix(circuit_edge_pairs)
    in_maps = []
    for c in range(N_CORES):
        shard = np.ascontiguousarray(
            P[c * BPC : (c + 1) * BPC, :L, :], dtype=np.float32
        )
        in_maps.append({"pq": pack_shard(shard), "cs": csb})
    return in_maps


def reduce_results(per_core_r, d_error):
    R = np.zeros((H, H), np.float64)
    for r in per_core_r:
        R += np.asarray(r).astype(np.float64)
    out = (np.asarray(d_error).astype(np.float64) * R).sum() / (
        2.0 * B * E * Q_SCALE * Q_SCALE
    )
    return np.array(out, dtype=np.float32)


def run_spmd(P, circuit_edge_pairs, **kwargs):
    """Run on the 8 NeuronCores; returns (list of per-core R, BassKernelResults)."""
    from concourse.bass_utils import run_bass_kernel_spmd

    nc = get_nc()
    in_maps = make_in_maps(P, circuit_edge_pairs)
    res = run_bass_kernel_spmd(nc, in_maps, core_ids=list(range(N_CORES)), **kwargs)
    per_core_r = [res.results[c]["r_out"] for c in range(N_CORES)]
    return per_core_r, res


def kernel(P, d_error, circuit_edge_pairs, num_logical):
    assert int(num_logical) == L
    per_core_r, _ = run_spmd(P, circuit_edge_pairs)
    return reduce_results(per_core_r, d_error)


# revision 13
# speedup vs baseline: 1.3675x; 1.2241x over previous
"""Trainium2 Bass kernel for nn_ErrorAwareEdgeLoss.

reference:  cost[b,e] = sum_{p,q} P[b,i_e,p] * d_error[p,q] * P[b,j_e,q]
            result    = mean_{b,e} cost[b,e]

The edge pairs only enter through the count matrix
    C2[l1,l2] = C + C^T,  C[l1,l2] = #edges with (i_e,j_e) == (l1,l2),
and since d_error is symmetric the result collapses to
    result = <d_error, sum_b Q_b^T C2 Q_b> / (2*B*E),
with Q_b = P[b,:64,:].  C2 has small integer entries (max ~6): exact in
bf16/fp8.

Device work per core (256 batches, data-parallel over batch):
  - HWDGE DMA of host-packed Q groups into [128, ncols] tiles
    (two batches stacked on the 128 partitions).
  - Y = blockdiag(C2,C2) @ Q: one K=128 matmul per 512-wide slab
    (the zero off-diagonal blocks cost nothing: matmul time = N cols).
  - PSUM->SBUF casts of Y, slab-granular, alternating DVE / ACT engines.
  - R += Q_pair^T @ Y_pair (K=128 = 2 batches, N=128) accumulated in
    PSUM f32; R-matmuls run one group behind Y so the PE never waits
    on the casts.
  - write per-core R (128x128 f32) to DRAM.
Host: R_total = sum_c R_c ; result = <d_error, R_total> / (2*B*E*S^2).
"""

import sys

_TRN_REPO = "/opt/trn_rl_repo"
if _TRN_REPO not in sys.path:
    sys.path.insert(0, _TRN_REPO)

import numpy as np
import ml_dtypes

B, L, H = 2048, 64, 128     # batch, logical qubits, physical dim
E = 512                     # number of circuit edges
N_CORES = 8
BPC = B // N_CORES          # 256 batches per core
SLAB = 512                  # matmul moving-operand width (= 4 pair-blocks)
NSLABS = BPC * L * H // (128 * SLAB)   # 32 slabs of 512 cols per core

# Q dtype: "bf16" (exact) or "fp8" (halves DMA traffic; rel err ~1.5e-3)
Q_DTYPE = "fp8"
Q_SCALE = 64.0 if Q_DTYPE == "fp8" else 1.0
# Y cast output dtype; "fp8" keeps all matmuls uniform-dtype (rel ~2.2e-3)
Y_DTYPE = "fp8"
# group sizes in 512-col slabs (sum must be NSLABS). Small leading groups
# start the PE early; bigger trailing groups amortize per-DMA overhead.
GROUP_SLABS = [1, 3, 4, 4, 4, 4, 6, 6]
# engine for each Y-slab PSUM->SBUF cast, round-robin: v=DVE, s=ACT, g=Pool
CAST_PATTERN = "vs"
# number of 512-col PE warmup matmuls issued before the first real slab
# (ramps the PE clock out of its low p-state while the first DMA lands)
WARMUP_MMS = 2

_CACHE = {}


def _np_qdt():
    return ml_dtypes.bfloat16 if Q_DTYPE == "bf16" else ml_dtypes.float8_e4m3fn


def _build():
    import concourse.tile as tile
    from concourse import bacc, mybir

    f32 = mybir.dt.float32
    bf16 = mybir.dt.bfloat16
    qdt = bf16 if Q_DTYPE == "bf16" else mybir.dt.float8e4
    ydt = bf16 if Y_DTYPE == "bf16" else mybir.dt.float8e4

    assert sum(GROUP_SLABS) == NSLABS

    nc = bacc.Bacc(None)
    # host-packed shard: row p holds, concatenated over (slab, pair-block),
    # Q[batch, p%64, :] for batch = 2*(col block) + p//64 — every group
    # load is a plain 2D DMA with a contiguous run per partition.
    pq = nc.dram_tensor("pq", [128, NSLABS * SLAB], qdt, kind="ExternalInput")
    # full block-diag(C2, C2), packed on host: one DMA, no memset needed
    cs = nc.dram_tensor("cs", [128, 128], qdt, kind="ExternalInput")
    r_out = nc.dram_tensor("r_out", [H, H], f32, kind="ExternalOutput")

    with tile.TileContext(nc) as tc:
        with (
            tc.tile_pool(name="singles", bufs=1) as singles,
            tc.tile_pool(name="qpool", bufs=6) as qpool,
            tc.tile_pool(name="ypool", bufs=5) as ypool,
            tc.tile_pool(name="yps", bufs=5, space="PSUM") as yps,
            tc.tile_pool(name="rps", bufs=1, space="PSUM") as rps,
        ):
            # csbd load off the SP queue (ACT issues it) so the group-0 load
            # and the csbd load generate their descriptors concurrently.
            csbd = singles.tile([128, 128], qdt)
            nc.scalar.dma_start(out=csbd[:, :], in_=cs[:, :])

            r_psum = rps.tile([128, H], f32)

            if WARMUP_MMS:
                warm = singles.tile([128, SLAB], qdt)
                nc.gpsimd.memset(warm[:, :], 0)
                wps = rps.tile([128, SLAB], f32)
                for _ in range(WARMUP_MMS):
                    nc.tensor.matmul(
                        wps[:, :], lhsT=warm[:, 0:128], rhs=warm[:, :],
                        start=True, stop=True, skip_group_check=True,
                    )

            _flags = {"first": True, "slab": 0}

            def emit_group(gi, c0, k):
                npairs = 4 * k
                qbf = qpool.tile([128, npairs, H], qdt)
                nc.sync.dma_start(out=qbf[:, :, :], in_=pq[:, c0 : c0 + k * SLAB])
                ybf = ypool.tile([128, npairs, H], ydt)
                for s in range(k):
                    yy = yps.tile([128, SLAB], f32)
                    nc.tensor.matmul(
                        yy[:, :], lhsT=csbd[:, :], rhs=qbf[:, 4 * s : 4 * s + 4, :],
                        start=True, stop=True, skip_group_check=True,
                    )
                    # PSUM -> SBUF cast, engines rotate per CAST_PATTERN
                    dst = ybf[:, 4 * s : 4 * s + 4, :]
                    eng = CAST_PATTERN[_flags["slab"] % len(CAST_PATTERN)]
                    _flags["slab"] += 1
                    if eng == "v":
                        nc.vector.tensor_copy(dst, yy[:, :])
                    elif eng == "s":
                        nc.scalar.copy(dst, yy[:, :])
                    else:
                        nc.gpsimd.tensor_copy(dst, yy[:, :])
                return qbf, ybf, npairs

            def emit_r(qbf, ybf, npairs, is_last_group):
                for pp in range(npairs):
                    first = _flags["first"]
                    _flags["first"] = False
                    last = is_last_group and pp == npairs - 1
                    nc.tensor.matmul(
                        r_psum[:, :],
                        lhsT=qbf[:, pp, :],
                        rhs=ybf[:, pp, :],
                        start=first, stop=last, skip_group_check=True,
                    )

            prev = None
            c0 = 0
            for gi, k in enumerate(GROUP_SLABS):
                cur = emit_group(gi, c0, k)
                c0 += k * SLAB
                if prev is not None:
                    emit_r(*prev, is_last_group=False)
                prev = cur
            emit_r(*prev, is_last_group=True)

            rsb = singles.tile([128, H], f32)
            nc.vector.tensor_copy(rsb[:, :], r_psum[:, :])
            nc.sync.dma_start(out=r_out[:, :], in_=rsb[:, :])

    nc.compile()
    return nc


def get_nc():
    key = ("nc", Q_DTYPE, Y_DTYPE, tuple(GROUP_SLABS), WARMUP_MMS, CAST_PATTERN)
    if key not in _CACHE:
        _CACHE[key] = _build()
    return _CACHE[key]


def make_count_matrix(circuit_edge_pairs):
    pairs = np.asarray(circuit_edge_pairs).astype(np.int64)
    C = np.zeros((L, L), np.float64)
    np.add.at(C, (pairs[:, 0], pairs[:, 1]), 1.0)
    C2 = C + C.T
    bd = np.zeros((128, 128), np.float64)
    bd[:L, :L] = C2
    bd[L:, L:] = C2
    out = bd.astype(_np_qdt())
    assert np.array_equal(out.astype(np.float64), bd), "C2 not exact in Q dtype"
    return out


def pack_shard(Q):
    """(256, 64, 128) f32 -> (128, 16384): partition p, col j*128+h holds
    Q[2*j + p//64, p%64, h] * Q_SCALE (pair-blocked along columns)."""
    arr = (Q * Q_SCALE).reshape(BPC // 2, 2, L, H)
    return np.ascontiguousarray(
        arr.transpose(1, 2, 0, 3).reshape(128, NSLABS * SLAB)
    ).astype(_np_qdt())


def make_in_maps(P, circuit_edge_pairs):
    P = np.asarray(P)
    csb = make_count_matrix(circuit_edge_pairs)
    in_maps = []
    for c in range(N_CORES):
        shard = np.ascontiguousarray(
            P[c * BPC : (c + 1) * BPC, :L, :], dtype=np.float32
        )
        in_maps.append({"pq": pack_shard(shard), "cs": csb})
    return in_maps


def reduce_results(per_core_r, d_error):
    R = np.zeros((H, H), np.float64)
    for r in per_core_r:
        R += np.asarray(r).astype(np.float64)
    out = (np.asarray(d_error).astype(np.float64) * R).sum() / (
        2.0 * B * E * Q_SCALE * Q_SCALE
    )
    return np.array(out, dtype=np.float32)


def run_spmd(P, circuit_edge_pairs, **kwargs):
    """Run on the 8 NeuronCores; returns (list of per-core R, BassKernelResults)."""
    from concourse.bass_utils import run_bass_kernel_spmd

    nc = get_nc()
    in_maps = make_in_maps(P, circuit_edge_pairs)
    res = run_bass_kernel_spmd(nc, in_maps, core_ids=list(range(N_CORES)), **kwargs)
    per_core_r = [res.results[c]["r_out"] for c in range(N_CORES)]
    return per_core_r, res


def kernel(P, d_error, circuit_edge_pairs, num_logical):
    assert int(num_logical) == L
    per_core_r, _ = run_spmd(P, circuit_edge_pairs)
    return reduce_results(per_core_r, d_error)


# revision 16
# speedup vs baseline: 1.4005x; 1.0241x over previous
"""Trainium2 Bass kernel for nn_ErrorAwareEdgeLoss.

reference:  cost[b,e] = sum_{p,q} P[b,i_e,p] * d_error[p,q] * P[b,j_e,q]
            result    = mean_{b,e} cost[b,e]

The edge pairs only enter through the count matrix
    C2[l1,l2] = C + C^T,  C[l1,l2] = #edges with (i_e,j_e) == (l1,l2),
and since d_error is symmetric the result collapses to
    result = <d_error, sum_b Q_b^T C2 Q_b> / (2*B*E),
with Q_b = P[b,:64,:].  C2 has small integer entries (max ~6): exact in
bf16/fp8.

Device work per core (256 batches, data-parallel over batch):
  - HWDGE DMA of host-packed Q groups into [128, ncols] tiles
    (two batches stacked on the 128 partitions).
  - Y = blockdiag(C2,C2) @ Q: one K=128 matmul per 512-wide slab
    (the zero off-diagonal blocks cost nothing: matmul time = N cols).
  - PSUM->SBUF casts of Y, slab-granular, alternating DVE / ACT engines.
  - R += Q_pair^T @ Y_pair (K=128 = 2 batches, N=128) accumulated in
    PSUM f32; R-matmuls run one group behind Y so the PE never waits
    on the casts.
  - write per-core R (128x128 f32) to DRAM.
Host: R_total = sum_c R_c ; result = <d_error, R_total> / (2*B*E*S^2).
"""

import sys

_TRN_REPO = "/opt/trn_rl_repo"
if _TRN_REPO not in sys.path:
    sys.path.insert(0, _TRN_REPO)

import numpy as np
import ml_dtypes

B, L, H = 2048, 64, 128     # batch, logical qubits, physical dim
E = 512                     # number of circuit edges
N_CORES = 8
BPC = B // N_CORES          # 256 batches per core
SLAB = 512                  # matmul moving-operand width (= 4 pair-blocks)
NSLABS = BPC * L * H // (128 * SLAB)   # 32 slabs of 512 cols per core

# Q dtype: "bf16" (exact) or "fp8" (halves DMA traffic; rel err ~1.5e-3)
Q_DTYPE = "fp8"
Q_SCALE = 64.0 if Q_DTYPE == "fp8" else 1.0
# Y cast output dtype; "fp8" keeps all matmuls uniform-dtype (rel ~2.2e-3)
Y_DTYPE = "fp8"
# group sizes in 512-col slabs (sum must be NSLABS). Small leading groups
# start the PE early; bigger trailing groups amortize per-DMA overhead.
GROUP_SLABS = [2, 4, 6, 6, 6, 8]
# engine for each Y-slab PSUM->SBUF cast, round-robin: v=DVE, s=ACT, g=Pool
CAST_PATTERN = "vs"
# number of 512-col PE warmup matmuls issued before the first real slab
# (ramps the PE clock out of its low p-state while the first DMA lands)
WARMUP_MMS = 2

_CACHE = {}


def _np_qdt():
    return ml_dtypes.bfloat16 if Q_DTYPE == "bf16" else ml_dtypes.float8_e4m3fn


def _build():
    import concourse.tile as tile
    from concourse import bacc, mybir

    f32 = mybir.dt.float32
    bf16 = mybir.dt.bfloat16
    qdt = bf16 if Q_DTYPE == "bf16" else mybir.dt.float8e4
    ydt = bf16 if Y_DTYPE == "bf16" else mybir.dt.float8e4

    assert sum(GROUP_SLABS) == NSLABS

    nc = bacc.Bacc(None)
    # host-packed shard: row p holds, concatenated over (slab, pair-block),
    # Q[batch, p%64, :] for batch = 2*(col block) + p//64 — every group
    # load is a plain 2D DMA with a contiguous run per partition.
    pq = nc.dram_tensor("pq", [128, NSLABS * SLAB], qdt, kind="ExternalInput")
    # full block-diag(C2, C2), packed on host: one DMA, no memset needed
    cs = nc.dram_tensor("cs", [128, 128], qdt, kind="ExternalInput")
    r_out = nc.dram_tensor("r_out", [H, H], f32, kind="ExternalOutput")

    with tile.TileContext(nc) as tc:
        with (
            tc.tile_pool(name="singles", bufs=1) as singles,
            tc.tile_pool(name="qpool", bufs=6) as qpool,
            tc.tile_pool(name="ypool", bufs=6) as ypool,
            tc.tile_pool(name="yps", bufs=3, space="PSUM") as yps,
            tc.tile_pool(name="rps", bufs=1, space="PSUM") as rps,
        ):
            # csbd load off the SP queue (ACT issues it) so the group-0 load
            # and the csbd load generate their descriptors concurrently.
            csbd = singles.tile([128, 128], qdt)
            nc.scalar.dma_start(out=csbd[:, :], in_=cs[:, :])

            r_psum = rps.tile([128, H], f32)

            if WARMUP_MMS:
                warm = singles.tile([128, SLAB], qdt)
                nc.gpsimd.memset(warm[:, :], 0)
                wps = rps.tile([128, SLAB], f32)
                for _ in range(WARMUP_MMS):
                    nc.tensor.matmul(
                        wps[:, :], lhsT=warm[:, 0:128], rhs=warm[:, :],
                        start=True, stop=True, skip_group_check=True,
                    )

            _flags = {"first": True, "slab": 0}

            def emit_group(gi, c0, k):
                npairs = 4 * k
                qbf = qpool.tile([128, npairs, H], qdt)
                nc.sync.dma_start(out=qbf[:, :, :], in_=pq[:, c0 : c0 + k * SLAB])
                ybf = ypool.tile([128, npairs, H], ydt)
                assert k % 2 == 0, "groups must be an even number of slabs"
                for c in range(k // 2):
                    # 2-slab (1024-col) PSUM chunk: two matmuls, one cast
                    yy = yps.tile([128, 2 * SLAB], f32)
                    for h in range(2):
                        s = 2 * c + h
                        nc.tensor.matmul(
                            yy[:, h * SLAB : (h + 1) * SLAB],
                            lhsT=csbd[:, :], rhs=qbf[:, 4 * s : 4 * s + 4, :],
                            start=True, stop=True, skip_group_check=True,
                        )
                    dst = ybf[:, 8 * c : 8 * c + 8, :]
                    eng = CAST_PATTERN[_flags["slab"] % len(CAST_PATTERN)]
                    _flags["slab"] += 1
                    if eng == "v":
                        nc.vector.tensor_copy(dst, yy[:, :])
                    else:
                        nc.scalar.copy(dst, yy[:, :])
                return qbf, ybf, npairs

            def emit_r(qbf, ybf, npairs, is_last_group):
                for pp in range(npairs):
                    first = _flags["first"]
                    _flags["first"] = False
                    last = is_last_group and pp == npairs - 1
                    nc.tensor.matmul(
                        r_psum[:, :],
                        lhsT=qbf[:, pp, :],
                        rhs=ybf[:, pp, :],
                        start=first, stop=last, skip_group_check=True,
                    )

            prev = None
            c0 = 0
            for gi, k in enumerate(GROUP_SLABS):
                cur = emit_group(gi, c0, k)
                c0 += k * SLAB
                if prev is not None:
                    emit_r(*prev, is_last_group=False)
                prev = cur
            emit_r(*prev, is_last_group=True)

            rsb = singles.tile([128, H], f32)
            nc.vector.tensor_copy(rsb[:, :], r_psum[:, :])
            nc.sync.dma_start(out=r_out[:, :], in_=rsb[:, :])

    nc.compile()
    return nc


def get_nc():
    key = ("nc", Q_DTYPE, Y_DTYPE, tuple(GROUP_SLABS), WARMUP_MMS, CAST_PATTERN)
    if key not in _CACHE:
        _CACHE[key] = _build()
    return _CACHE[key]


def make_count_matrix(circuit_edge_pairs):
    pairs = np.asarray(circuit_edge_pairs).astype(np.int64)
    C = np.zeros((L, L), np.float64)
    np.add.at(C, (pairs[:, 0], pairs[:, 1]), 1.0)
    C2 = C + C.T
    bd = np.zeros((128, 128), np.float64)
    bd[:L, :L] = C2
    bd[L:, L:] = C2
    out = bd.astype(_np_qdt())
    assert np.array_equal(out.astype(np.float64), bd), "C2 not exact in Q dtype"
    return out


def pack_shard(Q):
    """(256, 64, 128) f32 -> (128, 16384): partition p, col j*128+h holds
    Q[2*j + p//64, p%64, h] * Q_SCALE (pair-blocked along columns)."""
    arr = (Q * Q_SCALE).reshape(BPC // 2, 2, L, H)
    return np.ascontiguousarray(
        arr.transpose(1, 2, 0, 3).reshape(128, NSLABS * SLAB)
    ).astype(_np_qdt())


def make_in_maps(P, circuit_edge_pairs):
    P = np.asarray(P)
    csb = make_count_matrix(circuit_edge_pairs)
    in_maps = []
    for c in range(N_CORES):
        shard = np.ascontiguousarray(
            P[c * BPC : (c + 1) * BPC, :L, :], dtype=np.float32
        )
        in_maps.append({"pq": pack_shard(shard), "cs": csb})
    return in_maps


def reduce_results(per_core_r, d_error):
    R = np.zeros((H, H), np.float64)
    for r in per_core_r:
        R += np.asarray(r).astype(np.float64)
    out = (np.asarray(d_error).astype(np.float64) * R).sum() / (
        2.0 * B * E * Q_SCALE * Q_SCALE
    )
    return np.array(out, dtype=np.float32)


def run_spmd(P, circuit_edge_pairs, **kwargs):
    """Run on the 8 NeuronCores; returns (list of per-core R, BassKernelResults)."""
    from concourse.bass_utils import run_bass_kernel_spmd

    nc = get_nc()
    in_maps = make_in_maps(P, circuit_edge_pairs)
    res = run_bass_kernel_spmd(nc, in_maps, core_ids=list(range(N_CORES)), **kwargs)
    per_core_r = [res.results[c]["r_out"] for c in range(N_CORES)]
    return per_core_r, res


def kernel(P, d_error, circuit_edge_pairs, num_logical):
    assert int(num_logical) == L
    per_core_r, _ = run_spmd(P, circuit_edge_pairs)
    return reduce_results(per_core_r, d_error)
